# revision 26
# baseline (speedup 1.0000x reference)
"""Trainium2 Bass kernel for nn_MultiHeadAttention_62766652064333.

Reference computation (per batch b, all 8 "heads" identical):
    Ql = Q @ Wq + bq;  Kl = K @ Wk + bk;  Vl = V @ Wv + bv
    scores = Ql @ Kl.T / sqrt(dm) + mask * (-1e9)
    att = softmax(scores, axis=-1)
    head = att @ Vl
    Y = tile(head, h) @ Wl + bl     == head @ Wlsum + bl   (identical heads)
    att_ws = broadcast att over h

Algebraic restructuring (host does weight-only preprocessing):
    M    = Wq @ Wk.T                so  Ql @ Kl.T = Q @ M @ K.T + rank-1 terms
    WVL  = Wv @ Wlsum               so  head @ Wlsum = att @ V @ WVL + bv-term
    u[k] = K @ (Wk @ bq)            the only bias term that survives softmax
                                    (bk- and const-terms are per-row constants,
                                     softmax is invariant to them)
    bl2  = bv @ Wlsum + bl          (rows of att sum to 1)

Sharding: data-parallel over batch — one batch per NeuronCore (8 cores).

Device dataflow (per core; PE contraction dim always on SBUF partitions,
no on-device transposes — host supplies QT/KT/VT = X[b].T):
    AT[do, q]   = sum_di M[di, do] QT[di, q]          32 MM
    Vl2[k, do]  = sum_di VT[di, k] WVL[di, do]        32 MM
    scoresT[k,q]= sum_do KT[do, k] AT[do, q]          64 MM
    exT         = Exp(scoresT/sqrt(dm) + mb[k])       ACT (mask+u bias)
    denom       = ones128.T @ exT                     16 MM (replicated rows)
    att         = exT * recip(denom)   -> f32 DMA (transposed; host undoes)
                                       -> bf16 att_n for the Y matmuls
    Y[q, :]     = sum_kt att_n[kt,q-block].T @ Vl2[kt] + bl2   64 MM

All tensor-engine operands are bfloat16 (FWL weight loads fully hidden).
"""

import numpy as np
import ml_dtypes
from contextlib import ExitStack

import concourse.bass as bass
import concourse.mybir as mybir
import concourse.tile as tile
from concourse import bacc
from concourse.bass_utils import run_bass_kernel_spmd

P = 128
DM = 512
H = 8
B = 8
SQ = 1024
SK = 1024
ND = DM // P     # 4 d-tiles of 128
NK = SK // P     # 8 k-tiles
NQ = SQ // P     # 8 q-tiles
NF = 512         # matmul moving free dim (one PSUM bank)
NH = SQ // NF    # 2 q-halves
F32 = mybir.dt.float32
BF16 = mybir.dt.bfloat16
SM_SCALE = float(1.0 / np.sqrt(np.float32(DM)))


WARMUP_MMS = 0


def build_bass(warmup_n=None):
    warmup_n = WARMUP_MMS if warmup_n is None else warmup_n
    nc = bacc.Bacc("TRN2", target_bir_lowering=False, debug=False)
    AF = mybir.ActivationFunctionType

    qt_d = nc.dram_tensor("qt", [DM, SQ], BF16, kind="ExternalInput").ap()
    kt_d = nc.dram_tensor("kt", [DM, SK], BF16, kind="ExternalInput").ap()
    vt_d = nc.dram_tensor("vt", [DM, SK], BF16, kind="ExternalInput").ap()
    m_d = nc.dram_tensor("m", [DM, DM], BF16, kind="ExternalInput").ap()
    wvl_d = nc.dram_tensor("wvl", [DM, DM], BF16, kind="ExternalInput").ap()
    bl_d = nc.dram_tensor("blr2", [P, DM], F32, kind="ExternalInput").ap()
    mb_d = nc.dram_tensor("mb", [P, NK], F32, kind="ExternalInput").ap()
    ones_d = nc.dram_tensor("ones", [P, P], BF16, kind="ExternalInput").ap()

    att_d = nc.dram_tensor("attT", [SK, SQ], F32, kind="ExternalOutput").ap()
    y_d = nc.dram_tensor("y", [SQ, DM], F32, kind="ExternalOutput").ap()

    with tile.TileContext(nc) as tc, ExitStack() as ctx:
        consts = ctx.enter_context(tc.tile_pool(name="consts", bufs=1))
        bigp = ctx.enter_context(tc.tile_pool(name="bigp", bufs=1))
        stage = ctx.enter_context(tc.tile_pool(name="stage", bufs=3))
        pwork = ctx.enter_context(tc.tile_pool(name="pwork", bufs=5, space="PSUM"))
        pden = ctx.enter_context(tc.tile_pool(name="pden", bufs=2, space="PSUM"))
        pwarm = ctx.enter_context(tc.tile_pool(name="pwarm", bufs=1, space="PSUM"))

        # --- tiles ---
        ones128 = consts.tile([P, P], BF16, name="ones128", tag="ones128")
        bl_sb = consts.tile([P, DM], F32, name="bl_sb", tag="bl_sb")
        mb_sb = consts.tile([P, NK], F32, name="mb_sb", tag="mb_sb")
        m_sb = consts.tile([P, ND, DM], BF16, name="m_sb", tag="m_sb")
        wvl_sb = consts.tile([P, ND, DM], BF16, name="wvl_sb", tag="wvl_sb")

        qt_sb = bigp.tile([P, ND, SQ], BF16, name="qt_sb", tag="qt_sb")
        kt_sb = bigp.tile([P, ND, SK], BF16, name="kt_sb", tag="kt_sb")
        vt_sb = bigp.tile([P, ND, SK], BF16, name="vt_sb", tag="vt_sb")
        at_sb = bigp.tile([P, ND, SQ], BF16, name="at_sb", tag="at_sb")
        vl = bigp.tile([P, NK, DM], BF16, name="vl", tag="vl")
        ex = bigp.tile([P, NK, SQ], BF16, name="ex", tag="ex")
        att_n = bigp.tile([P, NK, SQ], BF16, name="att_n", tag="att_n")
        rc = consts.tile([P, SQ], F32, name="rc", tag="rc")
        rcb = consts.tile([P, SQ], BF16, name="rcb", tag="rcb")

        # --- input DMAs, ordered by first use, split per d-block; sync and
        # scalar issue to distinct HWDGE rings (FIFO per ring).
        m_r = m_d.rearrange("(o p) f -> p o f", p=P)
        wvl_r = wvl_d.rearrange("(o p) f -> p o f", p=P)
        qt_r = qt_d.rearrange("(o p) q -> p o q", p=P)
        kt_r = kt_d.rearrange("(o p) q -> p o q", p=P)
        vt_r = vt_d.rearrange("(o p) q -> p o q", p=P)

        # first-needed blocks race down both HWDGE rings in parallel
        for di in (0, 1):
            nc.sync.dma_start(m_sb[:, di, :], m_r[:, di, :])
            nc.sync.dma_start(qt_sb[:, di, :], qt_r[:, di, :])
        for di in (2, 3):
            nc.scalar.dma_start(m_sb[:, di, :], m_r[:, di, :])
            nc.scalar.dma_start(qt_sb[:, di, :], qt_r[:, di, :])
        for di in (0, 1):
            nc.sync.dma_start(kt_sb[:, di, :], kt_r[:, di, :])
        for di in (2, 3):
            nc.scalar.dma_start(kt_sb[:, di, :], kt_r[:, di, :])
        nc.scalar.dma_start(mb_sb[:], mb_d[:])
        nc.scalar.dma_start(ones128[:], ones_d[:])
        for di in (0, 1):
            nc.sync.dma_start(vt_sb[:, di, :], vt_r[:, di, :])
            nc.sync.dma_start(wvl_sb[:, di, :], wvl_r[:, di, :])
        for di in (2, 3):
            nc.scalar.dma_start(vt_sb[:, di, :], vt_r[:, di, :])
            nc.scalar.dma_start(wvl_sb[:, di, :], wvl_r[:, di, :])
        nc.scalar.dma_start(bl_sb[:], bl_d[:])

        # accumulation order follows DMA arrival (rings fill 0,2 then 1,3)
        DI_ORDER = (0, 2, 1, 3)

        def ps_tile(name):
            return pwork.tile([P, NF], F32, name=name, tag="ps")

        # --- PE warm-up: the first ~13us are DMA-bound and the PE HAM
        # clock-gate needs ~3.4us of sustained activity to reach 2.4GHz.
        # Burn the idle window on throwaway matmuls over a memset tile so
        # the real matmul stream starts (and stays) warm.
        if warmup_n:
            scratch = consts.tile([P, NF], BF16, name="scratch", tag="scratch")
            nc.vector.memset(scratch[:], 0.0)
            ps_warm = pwarm.tile([P, NF], F32, name="ps_warm", tag="ps_warm")
            for _ in range(warmup_n):
                nc.tensor.matmul(
                    ps_warm[:], scratch[:, 0:P], scratch[:], start=True, stop=True
                )

        # --- Phase A: AT = M.T @ QT ---
        for dt in range(ND):
            pss = [ps_tile(f"psat_{dt}_{qh}") for qh in range(NH)]
            for j, di in enumerate(DI_ORDER):
                for qh in range(NH):
                    nc.tensor.matmul(
                        pss[qh][:],
                        m_sb[:, di, dt * P:(dt + 1) * P],
                        qt_sb[:, di, qh * NF:(qh + 1) * NF],
                        start=(j == 0),
                        stop=(j == ND - 1),
                    )
            for qh in range(NH):
                nc.scalar.activation(
                    at_sb[:, dt, qh * NF:(qh + 1) * NF], pss[qh][:], AF.Copy
                )

        # --- Phase B: scoresT -> exp -> denominator ---
        pd = [
            pden.tile([P, NF], F32, name=f"pd_{qh}", tag="pden") for qh in range(NH)
        ]
        for kt_i in range(NK):
            pss = [ps_tile(f"pssc_{kt_i}_{qh}") for qh in range(NH)]
            for j, di in enumerate(DI_ORDER):
                for qh in range(NH):
                    nc.tensor.matmul(
                        pss[qh][:],
                        kt_sb[:, di, kt_i * P:(kt_i + 1) * P],
                        at_sb[:, di, qh * NF:(qh + 1) * NF],
                        start=(j == 0),
                        stop=(j == ND - 1),
                    )
            for qh in range(NH):
                qs = slice(qh * NF, (qh + 1) * NF)
                nc.scalar.activation(
                    ex[:, kt_i, qs],
                    pss[qh][:],
                    AF.Exp,
                    bias=mb_sb[:, kt_i:kt_i + 1],
                    scale=SM_SCALE,
                )
                nc.tensor.matmul(
                    pd[qh][:],
                    ones128[:],
                    ex[:, kt_i, qs],
                    start=(kt_i == 0),
                    stop=(kt_i == NK - 1),
                )

        # --- Phase A' (placed here so the PE fills the recip bubble):
        # Vl2 = VT.T @ WVL ---
        for kt_i in range(NK):
            ps = ps_tile(f"psvl_{kt_i}")
            for j, di in enumerate(DI_ORDER):
                nc.tensor.matmul(
                    ps[:],
                    vt_sb[:, di, kt_i * P:(kt_i + 1) * P],
                    wvl_sb[:, di, :],
                    start=(j == 0),
                    stop=(j == ND - 1),
                )
            nc.scalar.activation(vl[:, kt_i, :], ps[:], AF.Copy)

        # --- reciprocal of denominator (replicated rows) + bf16 copy so the
        # normalize muls run in the DVE 16-bit 2x mode ---
        for qh in range(NH):
            nc.vector.reciprocal(rc[:, qh * NF:(qh + 1) * NF], pd[qh][:])
            nc.vector.tensor_copy(
                out=rcb[:, qh * NF:(qh + 1) * NF], in_=rc[:, qh * NF:(qh + 1) * NF]
            )

        # --- normalize att (bf16, feeds Y); att output leaves via a casting
        # gpsimd DMA (bf16 -> f32), no f32 staging pass needed ---
        for kt_i in range(NK):
            for qh in range(NH):
                qs = slice(qh * NF, (qh + 1) * NF)
                nc.vector.tensor_mul(
                    out=att_n[:, kt_i, qs], in0=ex[:, kt_i, qs], in1=rcb[:, qs]
                )
            nc.gpsimd.dma_start(
                att_d[kt_i * P:(kt_i + 1) * P, :], att_n[:, kt_i, :]
            )

        # --- Phase Y: Y[q, :] = sum_kt att_n[kt].T @ Vl2[kt] + bl2 ---
        for qi in range(NQ):
            ps = ps_tile(f"psy_{qi}")
            for kt_i in range(NK):
                nc.tensor.matmul(
                    ps[:],
                    att_n[:, kt_i, qi * P:(qi + 1) * P],
                    vl[:, kt_i, :],
                    start=(kt_i == 0),
                    stop=(kt_i == NK - 1),
                )
            y_sb = stage.tile([P, DM], F32, name=f"y_sb_{qi}", tag="y_sb")
            nc.vector.tensor_add(out=y_sb[:], in0=ps[:], in1=bl_sb[:])
            nc.sync.dma_start(y_d[qi * P:(qi + 1) * P, :], y_sb[:])

    nc.compile()
    return nc


_NC_CACHE = {}


def get_nc():
    if "nc" not in _NC_CACHE:
        _NC_CACHE["nc"] = build_bass()
    return _NC_CACHE["nc"]


def prepare_in_maps(Q, K, V, mask, Wq, bq, Wk, bk, Wv, bv, Wl, bl):
    f = lambda a: np.ascontiguousarray(np.asarray(a, dtype=np.float32))
    Q, K, V = f(Q), f(K), f(V)
    Wq, Wk, Wv, Wl = f(Wq), f(Wk), f(Wv), f(Wl)
    bq, bk, bv, bl = f(bq), f(bk), f(bv), f(bl)
    mask = np.asarray(mask)

    bf = ml_dtypes.bfloat16
    g = lambda a: np.ascontiguousarray(a.astype(bf))

    wls = Wl.reshape(H, DM, DM).sum(axis=0, dtype=np.float64)
    m = (Wq.astype(np.float64) @ Wk.astype(np.float64).T).astype(np.float32)
    wvl = (Wv.astype(np.float64) @ wls).astype(np.float32)
    bl2 = (bv.astype(np.float64) @ wls + bl).astype(np.float32)
    blr2 = np.ascontiguousarray(np.broadcast_to(bl2, (P, DM)))
    wkbq = Wk @ bq  # [512]; u = K @ wkbq is the only surviving bias term

    in_maps = []
    for b in range(B):
        u = K[b] @ wkbq                                   # [1024]
        mb = mask[b, 0].astype(np.float32) * np.float32(-1e9) \
            + np.float32(SM_SCALE) * u
        in_maps.append(
            {
                "qt": g(Q[b].T),
                "kt": g(K[b].T),
                "vt": g(V[b].T),
                "m": g(m),
                "wvl": g(wvl),
                "blr2": blr2,
                "mb": np.ascontiguousarray(mb.reshape(NK, P).T),  # [128, 8]
                "ones": np.ones((P, P), dtype=bf),
            }
        )
    return in_maps


def postprocess(results):
    Y = np.stack([np.asarray(results[b]["y"]) for b in range(B)])
    att = np.stack([np.asarray(results[b]["attT"]).T for b in range(B)])
    att_ws = np.broadcast_to(att[:, None], (B, H, SQ, SK))
    return Y, att_ws


def kernel(Q, K, V, mask, Wq, bq, Wk, bk, Wv, bv, Wl, bl):
    nc = get_nc()
    in_maps = prepare_in_maps(Q, K, V, mask, Wq, bq, Wk, bk, Wv, bv, Wl, bl)
    res = run_bass_kernel_spmd(nc, in_maps, list(range(B)))
    return postprocess(res.results)


# revision 30
# speedup vs baseline: 1.0333x; 1.0333x over previous
"""Trainium2 Bass kernel for nn_MultiHeadAttention_62766652064333.

Reference computation (per batch b, all 8 "heads" identical):
    Ql = Q @ Wq + bq;  Kl = K @ Wk + bk;  Vl = V @ Wv + bv
    scores = Ql @ Kl.T / sqrt(dm) + mask * (-1e9)
    att = softmax(scores, axis=-1)
    head = att @ Vl
    Y = tile(head, h) @ Wl + bl     == head @ Wlsum + bl   (identical heads)
    att_ws = broadcast att over h

Algebraic restructuring (host does weight-only preprocessing):
    M    = Wq @ Wk.T                so  Ql @ Kl.T = Q @ M @ K.T + rank-1 terms
    WVL  = Wv @ Wlsum               so  head @ Wlsum = att @ V @ WVL + bv-term
    u[k] = K @ (Wk @ bq)            the only bias term that survives softmax
                                    (bk- and const-terms are per-row constants,
                                     softmax is invariant to them)
    bl2  = bv @ Wlsum + bl          (rows of att sum to 1)

Sharding: data-parallel over batch — one batch per NeuronCore (8 cores).

Device dataflow (per core; PE contraction dim always on SBUF partitions,
no on-device transposes — host supplies QT/KT/VT = X[b].T):
    AT[do, q]   = sum_di M[di, do] QT[di, q]          32 MM
    Vl2[k, do]  = sum_di VT[di, k] WVL[di, do]        32 MM
    scoresT[k,q]= sum_do KT[do, k] AT[do, q]          64 MM
    exT         = Exp(scoresT/sqrt(dm) + mb[k])       ACT (mask+u bias)
    denom       = ones128.T @ exT                     16 MM (replicated rows)
    att         = exT * recip(denom)   -> f32 DMA (transposed; host undoes)
                                       -> bf16 att_n for the Y matmuls
    Y[q, :]     = sum_kt att_n[kt,q-block].T @ Vl2[kt] + bl2   64 MM

All tensor-engine operands are bfloat16 (FWL weight loads fully hidden).
"""

import numpy as np
import ml_dtypes
from contextlib import ExitStack

import concourse.bass as bass
import concourse.mybir as mybir
import concourse.tile as tile
from concourse import bacc
from concourse.bass_utils import run_bass_kernel_spmd

P = 128
DM = 512
H = 8
B = 8
SQ = 1024
SK = 1024
ND = DM // P     # 4 d-tiles of 128
NK = SK // P     # 8 k-tiles
NQ = SQ // P     # 8 q-tiles
NF = 512         # matmul moving free dim (one PSUM bank)
NH = SQ // NF    # 2 q-halves
F32 = mybir.dt.float32
BF16 = mybir.dt.bfloat16
SM_SCALE = float(1.0 / np.sqrt(np.float32(DM)))


WARMUP_MMS = 0


def build_bass(warmup_n=None):
    warmup_n = WARMUP_MMS if warmup_n is None else warmup_n
    nc = bacc.Bacc("TRN2", target_bir_lowering=False, debug=False)
    AF = mybir.ActivationFunctionType

    # inputs come pre-packed in the SBUF layout ([partition, d-block, free])
    # so every DMA reads long contiguous per-partition lines at full rate
    qt_d = nc.dram_tensor("qt", [P, ND, SQ], BF16, kind="ExternalInput").ap()
    kt_d = nc.dram_tensor("kt", [P, ND, SK], BF16, kind="ExternalInput").ap()
    vt_d = nc.dram_tensor("vt", [P, ND, SK], BF16, kind="ExternalInput").ap()
    m_d = nc.dram_tensor("m", [P, ND, DM], BF16, kind="ExternalInput").ap()
    wvl_d = nc.dram_tensor("wvl", [P, ND, DM], BF16, kind="ExternalInput").ap()
    bl_d = nc.dram_tensor("blr2", [P, DM], F32, kind="ExternalInput").ap()
    mb_d = nc.dram_tensor("mb", [P, NK], F32, kind="ExternalInput").ap()
    ones_d = nc.dram_tensor("ones", [P, P], BF16, kind="ExternalInput").ap()

    att_d = nc.dram_tensor("attT", [SK, SQ], F32, kind="ExternalOutput").ap()
    y_d = nc.dram_tensor("y", [SQ, DM], F32, kind="ExternalOutput").ap()

    with tile.TileContext(nc) as tc, ExitStack() as ctx:
        consts = ctx.enter_context(tc.tile_pool(name="consts", bufs=1))
        bigp = ctx.enter_context(tc.tile_pool(name="bigp", bufs=1))
        stage = ctx.enter_context(tc.tile_pool(name="stage", bufs=3))
        pwork = ctx.enter_context(tc.tile_pool(name="pwork", bufs=5, space="PSUM"))
        pden = ctx.enter_context(tc.tile_pool(name="pden", bufs=2, space="PSUM"))
        pwarm = ctx.enter_context(tc.tile_pool(name="pwarm", bufs=1, space="PSUM"))

        # --- tiles ---
        ones128 = consts.tile([P, P], BF16, name="ones128", tag="ones128")
        bl_sb = consts.tile([P, DM], F32, name="bl_sb", tag="bl_sb")
        mb_sb = consts.tile([P, NK], F32, name="mb_sb", tag="mb_sb")
        m_sb = consts.tile([P, ND, DM], BF16, name="m_sb", tag="m_sb")
        wvl_sb = consts.tile([P, ND, DM], BF16, name="wvl_sb", tag="wvl_sb")

        qt_sb = bigp.tile([P, ND, SQ], BF16, name="qt_sb", tag="qt_sb")
        kt_sb = bigp.tile([P, ND, SK], BF16, name="kt_sb", tag="kt_sb")
        vt_sb = bigp.tile([P, ND, SK], BF16, name="vt_sb", tag="vt_sb")
        at_sb = bigp.tile([P, ND, SQ], BF16, name="at_sb", tag="at_sb")
        vl = bigp.tile([P, NK, DM], BF16, name="vl", tag="vl")
        ex = bigp.tile([P, NK, SQ], BF16, name="ex", tag="ex")
        att_n = bigp.tile([P, NK, SQ], BF16, name="att_n", tag="att_n")
        rc = consts.tile([P, SQ], F32, name="rc", tag="rc")
        rcb = consts.tile([P, SQ], BF16, name="rcb", tag="rcb")

        # Inputs arrive pre-packed in SBUF layout: every transfer is long
        # contiguous per-partition lines (full DMA rate, few issues).
        # sync and scalar issue to distinct HWDGE FIFO rings in parallel.
        nc.sync.dma_start(m_sb[:], m_d[:])
        nc.sync.dma_start(qt_sb[:, 0:2, :], qt_d[:, 0:2, :])
        nc.sync.dma_start(qt_sb[:, 2:4, :], qt_d[:, 2:4, :])
        nc.sync.dma_start(kt_sb[:, 0:2, :], kt_d[:, 0:2, :])
        nc.sync.dma_start(kt_sb[:, 2:4, :], kt_d[:, 2:4, :])
        nc.scalar.dma_start(mb_sb[:], mb_d[:])
        nc.scalar.dma_start(ones128[:], ones_d[:])
        nc.scalar.dma_start(vt_sb[:], vt_d[:])
        nc.scalar.dma_start(wvl_sb[:], wvl_d[:])
        nc.scalar.dma_start(bl_sb[:], bl_d[:])

        DI_ORDER = (0, 1, 2, 3)

        def ps_tile(name):
            return pwork.tile([P, NF], F32, name=name, tag="ps")

        # --- PE warm-up: the first ~13us are DMA-bound and the PE HAM
        # clock-gate needs ~3.4us of sustained activity to reach 2.4GHz.
        # Burn the idle window on throwaway matmuls over a memset tile so
        # the real matmul stream starts (and stays) warm.
        if warmup_n:
            scratch = consts.tile([P, NF], BF16, name="scratch", tag="scratch")
            nc.vector.memset(scratch[:], 0.0)
            ps_warm = pwarm.tile([P, NF], F32, name="ps_warm", tag="ps_warm")
            for _ in range(warmup_n):
                nc.tensor.matmul(
                    ps_warm[:], scratch[:, 0:P], scratch[:], start=True, stop=True
                )

        # --- Phase A: AT = M.T @ QT ---
        for dt in range(ND):
            pss = [ps_tile(f"psat_{dt}_{qh}") for qh in range(NH)]
            for j, di in enumerate(DI_ORDER):
                for qh in range(NH):
                    nc.tensor.matmul(
                        pss[qh][:],
                        m_sb[:, di, dt * P:(dt + 1) * P],
                        qt_sb[:, di, qh * NF:(qh + 1) * NF],
                        start=(j == 0),
                        stop=(j == ND - 1),
                    )
            for qh in range(NH):
                nc.scalar.activation(
                    at_sb[:, dt, qh * NF:(qh + 1) * NF], pss[qh][:], AF.Copy
                )

        # --- Phase B: scoresT -> exp -> denominator ---
        pd = [
            pden.tile([P, NF], F32, name=f"pd_{qh}", tag="pden") for qh in range(NH)
        ]
        for kt_i in range(NK):
            pss = [ps_tile(f"pssc_{kt_i}_{qh}") for qh in range(NH)]
            for j, di in enumerate(DI_ORDER):
                for qh in range(NH):
                    nc.tensor.matmul(
                        pss[qh][:],
                        kt_sb[:, di, kt_i * P:(kt_i + 1) * P],
                        at_sb[:, di, qh * NF:(qh + 1) * NF],
                        start=(j == 0),
                        stop=(j == ND - 1),
                    )
            for qh in range(NH):
                qs = slice(qh * NF, (qh + 1) * NF)
                nc.scalar.activation(
                    ex[:, kt_i, qs],
                    pss[qh][:],
                    AF.Exp,
                    bias=mb_sb[:, kt_i:kt_i + 1],
                    scale=SM_SCALE,
                )
                nc.tensor.matmul(
                    pd[qh][:],
                    ones128[:],
                    ex[:, kt_i, qs],
                    start=(kt_i == 0),
                    stop=(kt_i == NK - 1),
                )

        # --- Phase A' (placed here so the PE fills the recip bubble):
        # Vl2 = VT.T @ WVL ---
        for kt_i in range(NK):
            ps = ps_tile(f"psvl_{kt_i}")
            for j, di in enumerate(DI_ORDER):
                nc.tensor.matmul(
                    ps[:],
                    vt_sb[:, di, kt_i * P:(kt_i + 1) * P],
                    wvl_sb[:, di, :],
                    start=(j == 0),
                    stop=(j == ND - 1),
                )
            nc.scalar.activation(vl[:, kt_i, :], ps[:], AF.Copy)

        # --- reciprocal of denominator (replicated rows) + bf16 copy so the
        # normalize muls run in the DVE 16-bit 2x mode ---
        for qh in range(NH):
            nc.vector.reciprocal(rc[:, qh * NF:(qh + 1) * NF], pd[qh][:])
            nc.vector.tensor_copy(
                out=rcb[:, qh * NF:(qh + 1) * NF], in_=rc[:, qh * NF:(qh + 1) * NF]
            )

        # --- normalize att (bf16, feeds Y); att output leaves via a casting
        # gpsimd DMA (bf16 -> f32), no f32 staging pass needed ---
        for kt_i in range(NK):
            for qh in range(NH):
                qs = slice(qh * NF, (qh + 1) * NF)
                nc.vector.tensor_mul(
                    out=att_n[:, kt_i, qs], in0=ex[:, kt_i, qs], in1=rcb[:, qs]
                )
            nc.gpsimd.dma_start(
                att_d[kt_i * P:(kt_i + 1) * P, :], att_n[:, kt_i, :]
            )

        # --- Phase Y: Y[q, :] = sum_kt att_n[kt].T @ Vl2[kt] + bl2 ---
        for qi in range(NQ):
            ps = ps_tile(f"psy_{qi}")
            for kt_i in range(NK):
                nc.tensor.matmul(
                    ps[:],
                    att_n[:, kt_i, qi * P:(qi + 1) * P],
                    vl[:, kt_i, :],
                    start=(kt_i == 0),
                    stop=(kt_i == NK - 1),
                )
            y_sb = stage.tile([P, DM], F32, name=f"y_sb_{qi}", tag="y_sb")
            nc.vector.tensor_add(out=y_sb[:], in0=ps[:], in1=bl_sb[:])
            nc.sync.dma_start(y_d[qi * P:(qi + 1) * P, :], y_sb[:])

    nc.compile()
    return nc


_NC_CACHE = {}


def get_nc():
    if "nc" not in _NC_CACHE:
        _NC_CACHE["nc"] = build_bass()
    return _NC_CACHE["nc"]


def prepare_in_maps(Q, K, V, mask, Wq, bq, Wk, bk, Wv, bv, Wl, bl):
    f = lambda a: np.ascontiguousarray(np.asarray(a, dtype=np.float32))
    Q, K, V = f(Q), f(K), f(V)
    Wq, Wk, Wv, Wl = f(Wq), f(Wk), f(Wv), f(Wl)
    bq, bk, bv, bl = f(bq), f(bk), f(bv), f(bl)
    mask = np.asarray(mask)

    bf = ml_dtypes.bfloat16
    g = lambda a: np.ascontiguousarray(a.astype(bf))

    def pack(a):
        """[DM, X] d-major -> SBUF layout [128, ND, X], bf16, contiguous."""
        x = a.shape[1]
        return np.ascontiguousarray(
            a.reshape(ND, P, x).transpose(1, 0, 2).astype(bf)
        )

    wls = Wl.reshape(H, DM, DM).sum(axis=0, dtype=np.float64)
    m = (Wq.astype(np.float64) @ Wk.astype(np.float64).T).astype(np.float32)
    wvl = (Wv.astype(np.float64) @ wls).astype(np.float32)
    bl2 = (bv.astype(np.float64) @ wls + bl).astype(np.float32)
    blr2 = np.ascontiguousarray(np.broadcast_to(bl2, (P, DM)))
    wkbq = Wk @ bq  # [512]; u = K @ wkbq is the only surviving bias term

    in_maps = []
    for b in range(B):
        u = K[b] @ wkbq                                   # [1024]
        mb = mask[b, 0].astype(np.float32) * np.float32(-1e9) \
            + np.float32(SM_SCALE) * u
        in_maps.append(
            {
                "qt": pack(Q[b].T),
                "kt": pack(K[b].T),
                "vt": pack(V[b].T),
                "m": pack(m),
                "wvl": pack(wvl),
                "blr2": blr2,
                "mb": np.ascontiguousarray(mb.reshape(NK, P).T),  # [128, 8]
                "ones": np.ones((P, P), dtype=bf),
            }
        )
    return in_maps


def postprocess(results):
    Y = np.stack([np.asarray(results[b]["y"]) for b in range(B)])
    att = np.stack([np.asarray(results[b]["attT"]).T for b in range(B)])
    att_ws = np.broadcast_to(att[:, None], (B, H, SQ, SK))
    return Y, att_ws


def kernel(Q, K, V, mask, Wq, bq, Wk, bk, Wv, bv, Wl, bl):
    nc = get_nc()
    in_maps = prepare_in_maps(Q, K, V, mask, Wq, bq, Wk, bk, Wv, bv, Wl, bl)
    res = run_bass_kernel_spmd(nc, in_maps, list(range(B)))
    return postprocess(res.results)


# revision 31
# speedup vs baseline: 1.0579x; 1.0237x over previous
"""Trainium2 Bass kernel for nn_MultiHeadAttention_62766652064333.

Reference computation (per batch b, all 8 "heads" identical):
    Ql = Q @ Wq + bq;  Kl = K @ Wk + bk;  Vl = V @ Wv + bv
    scores = Ql @ Kl.T / sqrt(dm) + mask * (-1e9)
    att = softmax(scores, axis=-1)
    head = att @ Vl
    Y = tile(head, h) @ Wl + bl     == head @ Wlsum + bl   (identical heads)
    att_ws = broadcast att over h

Algebraic restructuring (host does weight-only preprocessing):
    M    = Wq @ Wk.T                so  Ql @ Kl.T = Q @ M @ K.T + rank-1 terms
    WVL  = Wv @ Wlsum               so  head @ Wlsum = att @ V @ WVL + bv-term
    u[k] = K @ (Wk @ bq)            the only bias term that survives softmax
                                    (bk- and const-terms are per-row constants,
                                     softmax is invariant to them)
    bl2  = bv @ Wlsum + bl          (rows of att sum to 1)

Sharding: data-parallel over batch — one batch per NeuronCore (8 cores).

Device dataflow (per core; PE contraction dim always on SBUF partitions,
no on-device transposes — host supplies QT/KT/VT = X[b].T):
    AT[do, q]   = sum_di M[di, do] QT[di, q]          32 MM
    Vl2[k, do]  = sum_di VT[di, k] WVL[di, do]        32 MM
    scoresT[k,q]= sum_do KT[do, k] AT[do, q]          64 MM
    exT         = Exp(scoresT/sqrt(dm) + mb[k])       ACT (mask+u bias)
    denom       = ones128.T @ exT                     16 MM (replicated rows)
    att         = exT * recip(denom)   -> f32 DMA (transposed; host undoes)
                                       -> bf16 att_n for the Y matmuls
    Y[q, :]     = sum_kt att_n[kt,q-block].T @ Vl2[kt] + bl2   64 MM

All tensor-engine operands are bfloat16 (FWL weight loads fully hidden).
"""

import numpy as np
import ml_dtypes
from contextlib import ExitStack

import concourse.bass as bass
import concourse.mybir as mybir
import concourse.tile as tile
from concourse import bacc
from concourse.bass_utils import run_bass_kernel_spmd

P = 128
DM = 512
H = 8
B = 8
SQ = 1024
SK = 1024
ND = DM // P     # 4 d-tiles of 128
NK = SK // P     # 8 k-tiles
NQ = SQ // P     # 8 q-tiles
NF = 512         # matmul moving free dim (one PSUM bank)
NH = SQ // NF    # 2 q-halves
F32 = mybir.dt.float32
BF16 = mybir.dt.bfloat16
SM_SCALE = float(1.0 / np.sqrt(np.float32(DM)))


WARMUP_MMS = 0


def build_bass(warmup_n=None):
    warmup_n = WARMUP_MMS if warmup_n is None else warmup_n
    nc = bacc.Bacc("TRN2", target_bir_lowering=False, debug=False)
    AF = mybir.ActivationFunctionType

    # inputs come pre-packed in the SBUF layout ([partition, d-block, free])
    # so every DMA reads long contiguous per-partition lines at full rate
    qt_d = nc.dram_tensor("qt", [P, ND, SQ], BF16, kind="ExternalInput").ap()
    kt_d = nc.dram_tensor("kt", [P, ND, SK], BF16, kind="ExternalInput").ap()
    vt_d = nc.dram_tensor("vt", [P, ND, SK], BF16, kind="ExternalInput").ap()
    m_d = nc.dram_tensor("m", [P, ND, DM], BF16, kind="ExternalInput").ap()
    wvl_d = nc.dram_tensor("wvl", [P, ND, DM], BF16, kind="ExternalInput").ap()
    bl_d = nc.dram_tensor("blr2", [P, DM], F32, kind="ExternalInput").ap()
    mb_d = nc.dram_tensor("mb", [P, NK], F32, kind="ExternalInput").ap()
    ones_d = nc.dram_tensor("ones", [P, P], BF16, kind="ExternalInput").ap()

    att_d = nc.dram_tensor("attT", [SK, SQ], F32, kind="ExternalOutput").ap()
    y_d = nc.dram_tensor("y", [SQ, DM], F32, kind="ExternalOutput").ap()

    with tile.TileContext(nc) as tc, ExitStack() as ctx:
        consts = ctx.enter_context(tc.tile_pool(name="consts", bufs=1))
        bigp = ctx.enter_context(tc.tile_pool(name="bigp", bufs=1))
        stage = ctx.enter_context(tc.tile_pool(name="stage", bufs=3))
        pwork = ctx.enter_context(tc.tile_pool(name="pwork", bufs=5, space="PSUM"))
        pden = ctx.enter_context(tc.tile_pool(name="pden", bufs=2, space="PSUM"))
        pwarm = ctx.enter_context(tc.tile_pool(name="pwarm", bufs=1, space="PSUM"))

        # --- tiles ---
        ones128 = consts.tile([P, P], BF16, name="ones128", tag="ones128")
        bl_sb = consts.tile([P, DM], F32, name="bl_sb", tag="bl_sb")
        mb_sb = consts.tile([P, NK], F32, name="mb_sb", tag="mb_sb")
        m_sb = consts.tile([P, ND, DM], BF16, name="m_sb", tag="m_sb")
        wvl_sb = consts.tile([P, ND, DM], BF16, name="wvl_sb", tag="wvl_sb")

        qt_sb = bigp.tile([P, ND, SQ], BF16, name="qt_sb", tag="qt_sb")
        kt_sb = bigp.tile([P, ND, SK], BF16, name="kt_sb", tag="kt_sb")
        vt_sb = bigp.tile([P, ND, SK], BF16, name="vt_sb", tag="vt_sb")
        at_sb = bigp.tile([P, ND, SQ], BF16, name="at_sb", tag="at_sb")
        vl = bigp.tile([P, NK, DM], BF16, name="vl", tag="vl")
        ex = bigp.tile([P, NK, SQ], BF16, name="ex", tag="ex")
        att_n = bigp.tile([P, NK, SQ], BF16, name="att_n", tag="att_n")
        rc = consts.tile([P, SQ], F32, name="rc", tag="rc")
        rcb = consts.tile([P, SQ], BF16, name="rcb", tag="rcb")

        # Inputs arrive pre-packed in SBUF layout: every transfer is long
        # contiguous per-partition lines (full DMA rate, few issues).
        # sync and scalar issue to distinct HWDGE FIFO rings in parallel.
        nc.sync.dma_start(m_sb[:], m_d[:])
        nc.sync.dma_start(qt_sb[:, 0:2, :], qt_d[:, 0:2, :])
        nc.scalar.dma_start(qt_sb[:, 2:4, :], qt_d[:, 2:4, :])
        nc.sync.dma_start(kt_sb[:, 0:2, :], kt_d[:, 0:2, :])
        nc.scalar.dma_start(kt_sb[:, 2:4, :], kt_d[:, 2:4, :])
        nc.scalar.dma_start(mb_sb[:], mb_d[:])
        nc.scalar.dma_start(ones128[:], ones_d[:])
        nc.sync.dma_start(vt_sb[:], vt_d[:])
        nc.scalar.dma_start(wvl_sb[:], wvl_d[:])
        nc.scalar.dma_start(bl_sb[:], bl_d[:])

        DI_ORDER = (0, 1, 2, 3)

        def ps_tile(name):
            return pwork.tile([P, NF], F32, name=name, tag="ps")

        # --- PE warm-up: the first ~13us are DMA-bound and the PE HAM
        # clock-gate needs ~3.4us of sustained activity to reach 2.4GHz.
        # Burn the idle window on throwaway matmuls over a memset tile so
        # the real matmul stream starts (and stays) warm.
        if warmup_n:
            scratch = consts.tile([P, NF], BF16, name="scratch", tag="scratch")
            nc.vector.memset(scratch[:], 0.0)
            ps_warm = pwarm.tile([P, NF], F32, name="ps_warm", tag="ps_warm")
            for _ in range(warmup_n):
                nc.tensor.matmul(
                    ps_warm[:], scratch[:, 0:P], scratch[:], start=True, stop=True
                )

        # --- Phase A: AT = M.T @ QT ---
        for dt in range(ND):
            pss = [ps_tile(f"psat_{dt}_{qh}") for qh in range(NH)]
            for j, di in enumerate(DI_ORDER):
                for qh in range(NH):
                    nc.tensor.matmul(
                        pss[qh][:],
                        m_sb[:, di, dt * P:(dt + 1) * P],
                        qt_sb[:, di, qh * NF:(qh + 1) * NF],
                        start=(j == 0),
                        stop=(j == ND - 1),
                    )
            for qh in range(NH):
                nc.scalar.activation(
                    at_sb[:, dt, qh * NF:(qh + 1) * NF], pss[qh][:], AF.Copy
                )

        # --- Phase B: scoresT -> exp -> denominator ---
        pd = [
            pden.tile([P, NF], F32, name=f"pd_{qh}", tag="pden") for qh in range(NH)
        ]
        for kt_i in range(NK):
            pss = [ps_tile(f"pssc_{kt_i}_{qh}") for qh in range(NH)]
            for j, di in enumerate(DI_ORDER):
                for qh in range(NH):
                    nc.tensor.matmul(
                        pss[qh][:],
                        kt_sb[:, di, kt_i * P:(kt_i + 1) * P],
                        at_sb[:, di, qh * NF:(qh + 1) * NF],
                        start=(j == 0),
                        stop=(j == ND - 1),
                    )
            for qh in range(NH):
                qs = slice(qh * NF, (qh + 1) * NF)
                nc.scalar.activation(
                    ex[:, kt_i, qs],
                    pss[qh][:],
                    AF.Exp,
                    bias=mb_sb[:, kt_i:kt_i + 1],
                    scale=SM_SCALE,
                )
                nc.tensor.matmul(
                    pd[qh][:],
                    ones128[:],
                    ex[:, kt_i, qs],
                    start=(kt_i == 0),
                    stop=(kt_i == NK - 1),
                )

        # --- Phase A' (placed here so the PE fills the recip bubble):
        # Vl2 = VT.T @ WVL ---
        for kt_i in range(NK):
            ps = ps_tile(f"psvl_{kt_i}")
            for j, di in enumerate(DI_ORDER):
                nc.tensor.matmul(
                    ps[:],
                    vt_sb[:, di, kt_i * P:(kt_i + 1) * P],
                    wvl_sb[:, di, :],
                    start=(j == 0),
                    stop=(j == ND - 1),
                )
            nc.scalar.activation(vl[:, kt_i, :], ps[:], AF.Copy)

        # --- reciprocal of denominator (replicated rows) + bf16 copy so the
        # normalize muls run in the DVE 16-bit 2x mode ---
        for qh in range(NH):
            nc.vector.reciprocal(rc[:, qh * NF:(qh + 1) * NF], pd[qh][:])
            nc.vector.tensor_copy(
                out=rcb[:, qh * NF:(qh + 1) * NF], in_=rc[:, qh * NF:(qh + 1) * NF]
            )

        # --- normalize att (bf16, feeds Y); att output leaves via a casting
        # gpsimd DMA (bf16 -> f32), no f32 staging pass needed ---
        for kt_i in range(NK):
            for qh in range(NH):
                qs = slice(qh * NF, (qh + 1) * NF)
                nc.vector.tensor_mul(
                    out=att_n[:, kt_i, qs], in0=ex[:, kt_i, qs], in1=rcb[:, qs]
                )
            nc.gpsimd.dma_start(
                att_d[kt_i * P:(kt_i + 1) * P, :], att_n[:, kt_i, :]
            )

        # --- Phase Y: Y[q, :] = sum_kt att_n[kt].T @ Vl2[kt] + bl2 ---
        for qi in range(NQ):
            ps = ps_tile(f"psy_{qi}")
            for kt_i in range(NK):
                nc.tensor.matmul(
                    ps[:],
                    att_n[:, kt_i, qi * P:(qi + 1) * P],
                    vl[:, kt_i, :],
                    start=(kt_i == 0),
                    stop=(kt_i == NK - 1),
                )
            y_sb = stage.tile([P, DM], F32, name=f"y_sb_{qi}", tag="y_sb")
            nc.vector.tensor_add(out=y_sb[:], in0=ps[:], in1=bl_sb[:])
            nc.sync.dma_start(y_d[qi * P:(qi + 1) * P, :], y_sb[:])

    nc.compile()
    return nc


_NC_CACHE = {}


def get_nc():
    if "nc" not in _NC_CACHE:
        _NC_CACHE["nc"] = build_bass()
    return _NC_CACHE["nc"]


def prepare_in_maps(Q, K, V, mask, Wq, bq, Wk, bk, Wv, bv, Wl, bl):
    f = lambda a: np.ascontiguousarray(np.asarray(a, dtype=np.float32))
    Q, K, V = f(Q), f(K), f(V)
    Wq, Wk, Wv, Wl = f(Wq), f(Wk), f(Wv), f(Wl)
    bq, bk, bv, bl = f(bq), f(bk), f(bv), f(bl)
    mask = np.asarray(mask)

    bf = ml_dtypes.bfloat16
    g = lambda a: np.ascontiguousarray(a.astype(bf))

    def pack(a):
        """[DM, X] d-major -> SBUF layout [128, ND, X], bf16, contiguous."""
        x = a.shape[1]
        return np.ascontiguousarray(
            a.reshape(ND, P, x).transpose(1, 0, 2).astype(bf)
        )

    wls = Wl.reshape(H, DM, DM).sum(axis=0, dtype=np.float64)
    m = (Wq.astype(np.float64) @ Wk.astype(np.float64).T).astype(np.float32)
    wvl = (Wv.astype(np.float64) @ wls).astype(np.float32)
    bl2 = (bv.astype(np.float64) @ wls + bl).astype(np.float32)
    blr2 = np.ascontiguousarray(np.broadcast_to(bl2, (P, DM)))
    wkbq = Wk @ bq  # [512]; u = K @ wkbq is the only surviving bias term

    in_maps = []
    for b in range(B):
        u = K[b] @ wkbq                                   # [1024]
        mb = mask[b, 0].astype(np.float32) * np.float32(-1e9) \
            + np.float32(SM_SCALE) * u
        in_maps.append(
            {
                "qt": pack(Q[b].T),
                "kt": pack(K[b].T),
                "vt": pack(V[b].T),
                "m": pack(m),
                "wvl": pack(wvl),
                "blr2": blr2,
                "mb": np.ascontiguousarray(mb.reshape(NK, P).T),  # [128, 8]
                "ones": np.ones((P, P), dtype=bf),
            }
        )
    return in_maps


def postprocess(results):
    Y = np.stack([np.asarray(results[b]["y"]) for b in range(B)])
    att = np.stack([np.asarray(results[b]["attT"]).T for b in range(B)])
    att_ws = np.broadcast_to(att[:, None], (B, H, SQ, SK))
    return Y, att_ws


def kernel(Q, K, V, mask, Wq, bq, Wk, bk, Wv, bv, Wl, bl):
    nc = get_nc()
    in_maps = prepare_in_maps(Q, K, V, mask, Wq, bq, Wk, bk, Wv, bv, Wl, bl)
    res = run_bass_kernel_spmd(nc, in_maps, list(range(B)))
    return postprocess(res.results)


# revision 32
# speedup vs baseline: 1.0837x; 1.0244x over previous
"""Trainium2 Bass kernel for nn_MultiHeadAttention_62766652064333.

Reference computation (per batch b, all 8 "heads" identical):
    Ql = Q @ Wq + bq;  Kl = K @ Wk + bk;  Vl = V @ Wv + bv
    scores = Ql @ Kl.T / sqrt(dm) + mask * (-1e9)
    att = softmax(scores, axis=-1)
    head = att @ Vl
    Y = tile(head, h) @ Wl + bl     == head @ Wlsum + bl   (identical heads)
    att_ws = broadcast att over h

Algebraic restructuring (host does weight-only preprocessing):
    M    = Wq @ Wk.T                so  Ql @ Kl.T = Q @ M @ K.T + rank-1 terms
    WVL  = Wv @ Wlsum               so  head @ Wlsum = att @ V @ WVL + bv-term
    u[k] = K @ (Wk @ bq)            the only bias term that survives softmax
                                    (bk- and const-terms are per-row constants,
                                     softmax is invariant to them)
    bl2  = bv @ Wlsum + bl          (rows of att sum to 1)

Sharding: data-parallel over batch — one batch per NeuronCore (8 cores).

Device dataflow (per core; PE contraction dim always on SBUF partitions,
no on-device transposes — host supplies QT/KT/VT = X[b].T):
    AT[do, q]   = sum_di M[di, do] QT[di, q]          32 MM
    Vl2[k, do]  = sum_di VT[di, k] WVL[di, do]        32 MM
    scoresT[k,q]= sum_do KT[do, k] AT[do, q]          64 MM
    exT         = Exp(scoresT/sqrt(dm) + mb[k])       ACT (mask+u bias)
    denom       = ones128.T @ exT                     16 MM (replicated rows)
    att         = exT * recip(denom)   -> f32 DMA (transposed; host undoes)
                                       -> bf16 att_n for the Y matmuls
    Y[q, :]     = sum_kt att_n[kt,q-block].T @ Vl2[kt] + bl2   64 MM

All tensor-engine operands are bfloat16 (FWL weight loads fully hidden).
"""

import numpy as np
import ml_dtypes
from contextlib import ExitStack

import concourse.bass as bass
import concourse.mybir as mybir
import concourse.tile as tile
from concourse import bacc
from concourse.bass_utils import run_bass_kernel_spmd

P = 128
DM = 512
H = 8
B = 8
SQ = 1024
SK = 1024
ND = DM // P     # 4 d-tiles of 128
NK = SK // P     # 8 k-tiles
NQ = SQ // P     # 8 q-tiles
NF = 512         # matmul moving free dim (one PSUM bank)
NH = SQ // NF    # 2 q-halves
F32 = mybir.dt.float32
BF16 = mybir.dt.bfloat16
SM_SCALE = float(1.0 / np.sqrt(np.float32(DM)))


WARMUP_MMS = 0


def build_bass(warmup_n=None):
    warmup_n = WARMUP_MMS if warmup_n is None else warmup_n
    nc = bacc.Bacc("TRN2", target_bir_lowering=False, debug=False)
    AF = mybir.ActivationFunctionType

    # inputs come pre-packed in the SBUF layout ([partition, d-block, free])
    # so every DMA reads long contiguous per-partition lines at full rate
    qt_d = nc.dram_tensor("qt", [P, ND, SQ], BF16, kind="ExternalInput").ap()
    kt_d = nc.dram_tensor("kt", [P, ND, SK], BF16, kind="ExternalInput").ap()
    vt_d = nc.dram_tensor("vt", [P, ND, SK], BF16, kind="ExternalInput").ap()
    m_d = nc.dram_tensor("m", [P, ND, DM], BF16, kind="ExternalInput").ap()
    wvl_d = nc.dram_tensor("wvl", [P, ND, DM], BF16, kind="ExternalInput").ap()
    bl_d = nc.dram_tensor("blr2", [P, DM], F32, kind="ExternalInput").ap()
    mb_d = nc.dram_tensor("mb", [P, NK], F32, kind="ExternalInput").ap()
    ones_d = nc.dram_tensor("ones", [P, P], BF16, kind="ExternalInput").ap()

    att_d = nc.dram_tensor("attT", [SK, SQ], F32, kind="ExternalOutput").ap()
    y_d = nc.dram_tensor("y", [SQ, DM], F32, kind="ExternalOutput").ap()

    with tile.TileContext(nc) as tc, ExitStack() as ctx:
        consts = ctx.enter_context(tc.tile_pool(name="consts", bufs=1))
        bigp = ctx.enter_context(tc.tile_pool(name="bigp", bufs=1))
        stage = ctx.enter_context(tc.tile_pool(name="stage", bufs=3))
        pwork = ctx.enter_context(tc.tile_pool(name="pwork", bufs=5, space="PSUM"))
        pden = ctx.enter_context(tc.tile_pool(name="pden", bufs=2, space="PSUM"))
        pwarm = ctx.enter_context(tc.tile_pool(name="pwarm", bufs=1, space="PSUM"))

        # --- tiles ---
        ones128 = consts.tile([P, P], BF16, name="ones128", tag="ones128")
        bl_sb = consts.tile([P, DM], F32, name="bl_sb", tag="bl_sb")
        mb_sb = consts.tile([P, NK], F32, name="mb_sb", tag="mb_sb")
        m_sb = consts.tile([P, ND, DM], BF16, name="m_sb", tag="m_sb")
        wvl_sb = consts.tile([P, ND, DM], BF16, name="wvl_sb", tag="wvl_sb")

        qt_sb = bigp.tile([P, ND, SQ], BF16, name="qt_sb", tag="qt_sb")
        kt_sb = bigp.tile([P, ND, SK], BF16, name="kt_sb", tag="kt_sb")
        vt_sb = bigp.tile([P, ND, SK], BF16, name="vt_sb", tag="vt_sb")
        at_sb = bigp.tile([P, ND, SQ], BF16, name="at_sb", tag="at_sb")
        vl = bigp.tile([P, NK, DM], BF16, name="vl", tag="vl")
        ex = bigp.tile([P, NK, SQ], BF16, name="ex", tag="ex")
        att_n = bigp.tile([P, NK, SQ], BF16, name="att_n", tag="att_n")
        rc = consts.tile([P, SQ], F32, name="rc", tag="rc")
        rcb = consts.tile([P, SQ], BF16, name="rcb", tag="rcb")

        # Inputs arrive pre-packed in SBUF layout: every transfer is long
        # contiguous per-partition lines (full DMA rate, few issues).
        # sync and scalar issue to distinct HWDGE FIFO rings in parallel.
        nc.sync.dma_start(m_sb[:, 0:2, :], m_d[:, 0:2, :])
        nc.scalar.dma_start(m_sb[:, 2:4, :], m_d[:, 2:4, :])
        nc.sync.dma_start(qt_sb[:, 0:2, :], qt_d[:, 0:2, :])
        nc.scalar.dma_start(qt_sb[:, 2:4, :], qt_d[:, 2:4, :])
        nc.sync.dma_start(kt_sb[:, 0:2, :], kt_d[:, 0:2, :])
        nc.scalar.dma_start(kt_sb[:, 2:4, :], kt_d[:, 2:4, :])
        nc.scalar.dma_start(mb_sb[:], mb_d[:])
        nc.scalar.dma_start(ones128[:], ones_d[:])
        nc.sync.dma_start(vt_sb[:], vt_d[:])
        nc.scalar.dma_start(wvl_sb[:], wvl_d[:])
        nc.scalar.dma_start(bl_sb[:], bl_d[:])

        DI_ORDER = (0, 1, 2, 3)

        def ps_tile(name):
            return pwork.tile([P, NF], F32, name=name, tag="ps")

        # --- PE warm-up: the first ~13us are DMA-bound and the PE HAM
        # clock-gate needs ~3.4us of sustained activity to reach 2.4GHz.
        # Burn the idle window on throwaway matmuls over a memset tile so
        # the real matmul stream starts (and stays) warm.
        if warmup_n:
            scratch = consts.tile([P, NF], BF16, name="scratch", tag="scratch")
            nc.vector.memset(scratch[:], 0.0)
            ps_warm = pwarm.tile([P, NF], F32, name="ps_warm", tag="ps_warm")
            for _ in range(warmup_n):
                nc.tensor.matmul(
                    ps_warm[:], scratch[:, 0:P], scratch[:], start=True, stop=True
                )

        # --- Phase A: AT = M.T @ QT ---
        for dt in range(ND):
            pss = [ps_tile(f"psat_{dt}_{qh}") for qh in range(NH)]
            for j, di in enumerate(DI_ORDER):
                for qh in range(NH):
                    nc.tensor.matmul(
                        pss[qh][:],
                        m_sb[:, di, dt * P:(dt + 1) * P],
                        qt_sb[:, di, qh * NF:(qh + 1) * NF],
                        start=(j == 0),
                        stop=(j == ND - 1),
                    )
            for qh in range(NH):
                nc.scalar.activation(
                    at_sb[:, dt, qh * NF:(qh + 1) * NF], pss[qh][:], AF.Copy
                )

        # --- Phase B: scoresT -> exp -> denominator ---
        pd = [
            pden.tile([P, NF], F32, name=f"pd_{qh}", tag="pden") for qh in range(NH)
        ]
        for kt_i in range(NK):
            pss = [ps_tile(f"pssc_{kt_i}_{qh}") for qh in range(NH)]
            for j, di in enumerate(DI_ORDER):
                for qh in range(NH):
                    nc.tensor.matmul(
                        pss[qh][:],
                        kt_sb[:, di, kt_i * P:(kt_i + 1) * P],
                        at_sb[:, di, qh * NF:(qh + 1) * NF],
                        start=(j == 0),
                        stop=(j == ND - 1),
                    )
            for qh in range(NH):
                qs = slice(qh * NF, (qh + 1) * NF)
                nc.scalar.activation(
                    ex[:, kt_i, qs],
                    pss[qh][:],
                    AF.Exp,
                    bias=mb_sb[:, kt_i:kt_i + 1],
                    scale=SM_SCALE,
                )
                nc.tensor.matmul(
                    pd[qh][:],
                    ones128[:],
                    ex[:, kt_i, qs],
                    start=(kt_i == 0),
                    stop=(kt_i == NK - 1),
                )

        # --- Phase A' (placed here so the PE fills the recip bubble):
        # Vl2 = VT.T @ WVL ---
        for kt_i in range(NK):
            ps = ps_tile(f"psvl_{kt_i}")
            for j, di in enumerate(DI_ORDER):
                nc.tensor.matmul(
                    ps[:],
                    vt_sb[:, di, kt_i * P:(kt_i + 1) * P],
                    wvl_sb[:, di, :],
                    start=(j == 0),
                    stop=(j == ND - 1),
                )
            nc.scalar.activation(vl[:, kt_i, :], ps[:], AF.Copy)

        # --- reciprocal of denominator (replicated rows) + bf16 copy so the
        # normalize muls run in the DVE 16-bit 2x mode ---
        for qh in range(NH):
            nc.vector.reciprocal(rc[:, qh * NF:(qh + 1) * NF], pd[qh][:])
            nc.vector.tensor_copy(
                out=rcb[:, qh * NF:(qh + 1) * NF], in_=rc[:, qh * NF:(qh + 1) * NF]
            )

        # --- normalize att (bf16, feeds Y); att output leaves via a casting
        # gpsimd DMA (bf16 -> f32), no f32 staging pass needed ---
        for kt_i in range(NK):
            for qh in range(NH):
                qs = slice(qh * NF, (qh + 1) * NF)
                nc.vector.tensor_mul(
                    out=att_n[:, kt_i, qs], in0=ex[:, kt_i, qs], in1=rcb[:, qs]
                )
            nc.gpsimd.dma_start(
                att_d[kt_i * P:(kt_i + 1) * P, :], att_n[:, kt_i, :]
            )

        # --- Phase Y: Y[q, :] = sum_kt att_n[kt].T @ Vl2[kt] + bl2 ---
        for qi in range(NQ):
            ps = ps_tile(f"psy_{qi}")
            for kt_i in range(NK):
                nc.tensor.matmul(
                    ps[:],
                    att_n[:, kt_i, qi * P:(qi + 1) * P],
                    vl[:, kt_i, :],
                    start=(kt_i == 0),
                    stop=(kt_i == NK - 1),
                )
            y_sb = stage.tile([P, DM], F32, name=f"y_sb_{qi}", tag="y_sb")
            nc.vector.tensor_add(out=y_sb[:], in0=ps[:], in1=bl_sb[:])
            nc.sync.dma_start(y_d[qi * P:(qi + 1) * P, :], y_sb[:])

    nc.compile()
    return nc


_NC_CACHE = {}


def get_nc():
    if "nc" not in _NC_CACHE:
        _NC_CACHE["nc"] = build_bass()
    return _NC_CACHE["nc"]


def prepare_in_maps(Q, K, V, mask, Wq, bq, Wk, bk, Wv, bv, Wl, bl):
    f = lambda a: np.ascontiguousarray(np.asarray(a, dtype=np.float32))
    Q, K, V = f(Q), f(K), f(V)
    Wq, Wk, Wv, Wl = f(Wq), f(Wk), f(Wv), f(Wl)
    bq, bk, bv, bl = f(bq), f(bk), f(bv), f(bl)
    mask = np.asarray(mask)

    bf = ml_dtypes.bfloat16
    g = lambda a: np.ascontiguousarray(a.astype(bf))

    def pack(a):
        """[DM, X] d-major -> SBUF layout [128, ND, X], bf16, contiguous."""
        x = a.shape[1]
        return np.ascontiguousarray(
            a.reshape(ND, P, x).transpose(1, 0, 2).astype(bf)
        )

    wls = Wl.reshape(H, DM, DM).sum(axis=0, dtype=np.float64)
    m = (Wq.astype(np.float64) @ Wk.astype(np.float64).T).astype(np.float32)
    wvl = (Wv.astype(np.float64) @ wls).astype(np.float32)
    bl2 = (bv.astype(np.float64) @ wls + bl).astype(np.float32)
    blr2 = np.ascontiguousarray(np.broadcast_to(bl2, (P, DM)))
    wkbq = Wk @ bq  # [512]; u = K @ wkbq is the only surviving bias term

    in_maps = []
    for b in range(B):
        u = K[b] @ wkbq                                   # [1024]
        mb = mask[b, 0].astype(np.float32) * np.float32(-1e9) \
            + np.float32(SM_SCALE) * u
        in_maps.append(
            {
                "qt": pack(Q[b].T),
                "kt": pack(K[b].T),
                "vt": pack(V[b].T),
                "m": pack(m),
                "wvl": pack(wvl),
                "blr2": blr2,
                "mb": np.ascontiguousarray(mb.reshape(NK, P).T),  # [128, 8]
                "ones": np.ones((P, P), dtype=bf),
            }
        )
    return in_maps


def postprocess(results):
    Y = np.stack([np.asarray(results[b]["y"]) for b in range(B)])
    att = np.stack([np.asarray(results[b]["attT"]).T for b in range(B)])
    att_ws = np.broadcast_to(att[:, None], (B, H, SQ, SK))
    return Y, att_ws


def kernel(Q, K, V, mask, Wq, bq, Wk, bk, Wv, bv, Wl, bl):
    nc = get_nc()
    in_maps = prepare_in_maps(Q, K, V, mask, Wq, bq, Wk, bk, Wv, bv, Wl, bl)
    res = run_bass_kernel_spmd(nc, in_maps, list(range(B)))
    return postprocess(res.results)


# revision 34
# speedup vs baseline: 1.0854x; 1.0016x over previous
"""Trainium2 Bass kernel for nn_MultiHeadAttention_62766652064333.

Reference computation (per batch b, all 8 "heads" identical):
    Ql = Q @ Wq + bq;  Kl = K @ Wk + bk;  Vl = V @ Wv + bv
    scores = Ql @ Kl.T / sqrt(dm) + mask * (-1e9)
    att = softmax(scores, axis=-1)
    head = att @ Vl
    Y = tile(head, h) @ Wl + bl     == head @ Wlsum + bl   (identical heads)
    att_ws = broadcast att over h

Algebraic restructuring (host does weight-only preprocessing):
    M    = Wq @ Wk.T                so  Ql @ Kl.T = Q @ M @ K.T + rank-1 terms
    WVL  = Wv @ Wlsum               so  head @ Wlsum = att @ V @ WVL + bv-term
    u[k] = K @ (Wk @ bq)            the only bias term that survives softmax
                                    (bk- and const-terms are per-row constants,
                                     softmax is invariant to them)
    bl2  = bv @ Wlsum + bl          (rows of att sum to 1)

Sharding: data-parallel over batch — one batch per NeuronCore (8 cores).

Device dataflow (per core; PE contraction dim always on SBUF partitions,
no on-device transposes — host supplies QT/KT/VT = X[b].T):
    AT[do, q]   = sum_di M[di, do] QT[di, q]          32 MM
    Vl2[k, do]  = sum_di VT[di, k] WVL[di, do]        32 MM
    scoresT[k,q]= sum_do KT[do, k] AT[do, q]          64 MM
    exT         = Exp(scoresT/sqrt(dm) + mb[k])       ACT (mask+u bias)
    denom       = ones128.T @ exT                     16 MM (replicated rows)
    att         = exT * recip(denom)   -> f32 DMA (transposed; host undoes)
                                       -> bf16 att_n for the Y matmuls
    Y[q, :]     = sum_kt att_n[kt,q-block].T @ Vl2[kt] + bl2   64 MM

All tensor-engine operands are bfloat16 (FWL weight loads fully hidden).
"""

import numpy as np
import ml_dtypes
from contextlib import ExitStack

import concourse.bass as bass
import concourse.mybir as mybir
import concourse.tile as tile
from concourse import bacc
from concourse.bass_utils import run_bass_kernel_spmd

P = 128
DM = 512
H = 8
B = 8
SQ = 1024
SK = 1024
ND = DM // P     # 4 d-tiles of 128
NK = SK // P     # 8 k-tiles
NQ = SQ // P     # 8 q-tiles
NF = 512         # matmul moving free dim (one PSUM bank)
NH = SQ // NF    # 2 q-halves
F32 = mybir.dt.float32
BF16 = mybir.dt.bfloat16
SM_SCALE = float(1.0 / np.sqrt(np.float32(DM)))


WARMUP_MMS = 0


def build_bass(warmup_n=None):
    warmup_n = WARMUP_MMS if warmup_n is None else warmup_n
    nc = bacc.Bacc("TRN2", target_bir_lowering=False, debug=False)
    AF = mybir.ActivationFunctionType

    # inputs come pre-packed in the SBUF layout ([partition, d-block, free])
    # so every DMA reads long contiguous per-partition lines at full rate
    qt_d = nc.dram_tensor("qt", [P, ND, SQ], BF16, kind="ExternalInput").ap()
    kt_d = nc.dram_tensor("kt", [P, ND, SK], BF16, kind="ExternalInput").ap()
    vt_d = nc.dram_tensor("vt", [P, ND, SK], BF16, kind="ExternalInput").ap()
    m_d = nc.dram_tensor("m", [P, ND, DM], BF16, kind="ExternalInput").ap()
    wvl_d = nc.dram_tensor("wvl", [P, ND, DM], BF16, kind="ExternalInput").ap()
    bl_d = nc.dram_tensor("blr2", [P, DM], F32, kind="ExternalInput").ap()
    mb_d = nc.dram_tensor("mb", [P, NK], F32, kind="ExternalInput").ap()
    ones_d = nc.dram_tensor("ones", [P, P], BF16, kind="ExternalInput").ap()

    att_d = nc.dram_tensor("attT", [SK, SQ], F32, kind="ExternalOutput").ap()
    y_d = nc.dram_tensor("y", [SQ, DM], F32, kind="ExternalOutput").ap()

    with tile.TileContext(nc) as tc, ExitStack() as ctx:
        consts = ctx.enter_context(tc.tile_pool(name="consts", bufs=1))
        bigp = ctx.enter_context(tc.tile_pool(name="bigp", bufs=1))
        stage = ctx.enter_context(tc.tile_pool(name="stage", bufs=3))
        pwork = ctx.enter_context(tc.tile_pool(name="pwork", bufs=5, space="PSUM"))
        pden = ctx.enter_context(tc.tile_pool(name="pden", bufs=2, space="PSUM"))
        pwarm = ctx.enter_context(tc.tile_pool(name="pwarm", bufs=1, space="PSUM"))

        # --- tiles ---
        ones128 = consts.tile([P, P], BF16, name="ones128", tag="ones128")
        bl_sb = consts.tile([P, DM], F32, name="bl_sb", tag="bl_sb")
        mb_sb = consts.tile([P, NK], F32, name="mb_sb", tag="mb_sb")
        m_sb = consts.tile([P, ND, DM], BF16, name="m_sb", tag="m_sb")
        wvl_sb = consts.tile([P, ND, DM], BF16, name="wvl_sb", tag="wvl_sb")

        qt_sb = bigp.tile([P, ND, SQ], BF16, name="qt_sb", tag="qt_sb")
        kt_sb = bigp.tile([P, ND, SK], BF16, name="kt_sb", tag="kt_sb")
        vt_sb = bigp.tile([P, ND, SK], BF16, name="vt_sb", tag="vt_sb")
        at_sb = bigp.tile([P, ND, SQ], BF16, name="at_sb", tag="at_sb")
        vl = bigp.tile([P, NK, DM], BF16, name="vl", tag="vl")
        ex = bigp.tile([P, NK, SQ], BF16, name="ex", tag="ex")
        att_n = bigp.tile([P, NK, SQ], BF16, name="att_n", tag="att_n")
        rc = consts.tile([P, SQ], F32, name="rc", tag="rc")
        rcb = consts.tile([P, SQ], BF16, name="rcb", tag="rcb")

        # Inputs arrive pre-packed in SBUF layout: every transfer is long
        # contiguous per-partition lines (full DMA rate, few issues).
        # sync and scalar issue to distinct HWDGE FIFO rings in parallel.
        nc.sync.dma_start(m_sb[:, 0:1, :], m_d[:, 0:1, :])
        nc.scalar.dma_start(m_sb[:, 2:3, :], m_d[:, 2:3, :])
        nc.sync.dma_start(qt_sb[:, 0:1, :], qt_d[:, 0:1, :])
        nc.scalar.dma_start(qt_sb[:, 2:3, :], qt_d[:, 2:3, :])
        nc.sync.dma_start(m_sb[:, 1:2, :], m_d[:, 1:2, :])
        nc.scalar.dma_start(m_sb[:, 3:4, :], m_d[:, 3:4, :])
        nc.sync.dma_start(qt_sb[:, 1:2, :], qt_d[:, 1:2, :])
        nc.scalar.dma_start(qt_sb[:, 3:4, :], qt_d[:, 3:4, :])
        nc.sync.dma_start(kt_sb[:, 0:2, :], kt_d[:, 0:2, :])
        nc.scalar.dma_start(kt_sb[:, 2:4, :], kt_d[:, 2:4, :])
        nc.scalar.dma_start(mb_sb[:], mb_d[:])
        nc.scalar.dma_start(ones128[:], ones_d[:])
        nc.sync.dma_start(vt_sb[:], vt_d[:])
        nc.scalar.dma_start(wvl_sb[:], wvl_d[:])
        nc.scalar.dma_start(bl_sb[:], bl_d[:])

        # consume d-blocks in DMA-arrival order (rings deliver 0,2 then 1,3)
        DI_ORDER = (0, 2, 1, 3)

        def ps_tile(name):
            return pwork.tile([P, NF], F32, name=name, tag="ps")

        # --- PE warm-up: the first ~13us are DMA-bound and the PE HAM
        # clock-gate needs ~3.4us of sustained activity to reach 2.4GHz.
        # Burn the idle window on throwaway matmuls over a memset tile so
        # the real matmul stream starts (and stays) warm.
        if warmup_n:
            scratch = consts.tile([P, NF], BF16, name="scratch", tag="scratch")
            nc.vector.memset(scratch[:], 0.0)
            ps_warm = pwarm.tile([P, NF], F32, name="ps_warm", tag="ps_warm")
            for _ in range(warmup_n):
                nc.tensor.matmul(
                    ps_warm[:], scratch[:, 0:P], scratch[:], start=True, stop=True
                )

        # --- Phase A: AT = M.T @ QT ---
        for dt in range(ND):
            pss = [ps_tile(f"psat_{dt}_{qh}") for qh in range(NH)]
            for j, di in enumerate(DI_ORDER):
                for qh in range(NH):
                    nc.tensor.matmul(
                        pss[qh][:],
                        m_sb[:, di, dt * P:(dt + 1) * P],
                        qt_sb[:, di, qh * NF:(qh + 1) * NF],
                        start=(j == 0),
                        stop=(j == ND - 1),
                    )
            for qh in range(NH):
                nc.scalar.activation(
                    at_sb[:, dt, qh * NF:(qh + 1) * NF], pss[qh][:], AF.Copy
                )

        # --- Phase B: scoresT -> exp -> denominator ---
        pd = [
            pden.tile([P, NF], F32, name=f"pd_{qh}", tag="pden") for qh in range(NH)
        ]
        for kt_i in range(NK):
            pss = [ps_tile(f"pssc_{kt_i}_{qh}") for qh in range(NH)]
            for j, di in enumerate(DI_ORDER):
                for qh in range(NH):
                    nc.tensor.matmul(
                        pss[qh][:],
                        kt_sb[:, di, kt_i * P:(kt_i + 1) * P],
                        at_sb[:, di, qh * NF:(qh + 1) * NF],
                        start=(j == 0),
                        stop=(j == ND - 1),
                    )
            for qh in range(NH):
                qs = slice(qh * NF, (qh + 1) * NF)
                nc.scalar.activation(
                    ex[:, kt_i, qs],
                    pss[qh][:],
                    AF.Exp,
                    bias=mb_sb[:, kt_i:kt_i + 1],
                    scale=SM_SCALE,
                )
                nc.tensor.matmul(
                    pd[qh][:],
                    ones128[:],
                    ex[:, kt_i, qs],
                    start=(kt_i == 0),
                    stop=(kt_i == NK - 1),
                )

        # --- Phase A' (placed here so the PE fills the recip bubble):
        # Vl2 = VT.T @ WVL ---
        for kt_i in range(NK):
            ps = ps_tile(f"psvl_{kt_i}")
            for j, di in enumerate(DI_ORDER):
                nc.tensor.matmul(
                    ps[:],
                    vt_sb[:, di, kt_i * P:(kt_i + 1) * P],
                    wvl_sb[:, di, :],
                    start=(j == 0),
                    stop=(j == ND - 1),
                )
            nc.scalar.activation(vl[:, kt_i, :], ps[:], AF.Copy)

        # --- reciprocal of denominator (replicated rows) + bf16 copy so the
        # normalize muls run in the DVE 16-bit 2x mode ---
        for qh in range(NH):
            nc.vector.reciprocal(rc[:, qh * NF:(qh + 1) * NF], pd[qh][:])
            nc.vector.tensor_copy(
                out=rcb[:, qh * NF:(qh + 1) * NF], in_=rc[:, qh * NF:(qh + 1) * NF]
            )

        # --- normalize att (bf16, feeds Y); att output leaves via a casting
        # gpsimd DMA (bf16 -> f32), no f32 staging pass needed ---
        for kt_i in range(NK):
            for qh in range(NH):
                qs = slice(qh * NF, (qh + 1) * NF)
                nc.vector.tensor_mul(
                    out=att_n[:, kt_i, qs], in0=ex[:, kt_i, qs], in1=rcb[:, qs]
                )
            nc.gpsimd.dma_start(
                att_d[kt_i * P:(kt_i + 1) * P, :], att_n[:, kt_i, :]
            )

        # --- Phase Y: Y[q, :] = sum_kt att_n[kt].T @ Vl2[kt] + bl2 ---
        for qi in range(NQ):
            ps = ps_tile(f"psy_{qi}")
            for kt_i in range(NK):
                nc.tensor.matmul(
                    ps[:],
                    att_n[:, kt_i, qi * P:(qi + 1) * P],
                    vl[:, kt_i, :],
                    start=(kt_i == 0),
                    stop=(kt_i == NK - 1),
                )
            y_sb = stage.tile([P, DM], F32, name=f"y_sb_{qi}", tag="y_sb")
            nc.vector.tensor_add(out=y_sb[:], in0=ps[:], in1=bl_sb[:])
            nc.sync.dma_start(y_d[qi * P:(qi + 1) * P, :], y_sb[:])

    nc.compile()
    return nc


_NC_CACHE = {}


def get_nc():
    if "nc" not in _NC_CACHE:
        _NC_CACHE["nc"] = build_bass()
    return _NC_CACHE["nc"]


def prepare_in_maps(Q, K, V, mask, Wq, bq, Wk, bk, Wv, bv, Wl, bl):
    f = lambda a: np.ascontiguousarray(np.asarray(a, dtype=np.float32))
    Q, K, V = f(Q), f(K), f(V)
    Wq, Wk, Wv, Wl = f(Wq), f(Wk), f(Wv), f(Wl)
    bq, bk, bv, bl = f(bq), f(bk), f(bv), f(bl)
    mask = np.asarray(mask)

    bf = ml_dtypes.bfloat16
    g = lambda a: np.ascontiguousarray(a.astype(bf))

    def pack(a):
        """[DM, X] d-major -> SBUF layout [128, ND, X], bf16, contiguous."""
        x = a.shape[1]
        return np.ascontiguousarray(
            a.reshape(ND, P, x).transpose(1, 0, 2).astype(bf)
        )

    wls = Wl.reshape(H, DM, DM).sum(axis=0, dtype=np.float64)
    m = (Wq.astype(np.float64) @ Wk.astype(np.float64).T).astype(np.float32)
    wvl = (Wv.astype(np.float64) @ wls).astype(np.float32)
    bl2 = (bv.astype(np.float64) @ wls + bl).astype(np.float32)
    blr2 = np.ascontiguousarray(np.broadcast_to(bl2, (P, DM)))
    wkbq = Wk @ bq  # [512]; u = K @ wkbq is the only surviving bias term

    in_maps = []
    for b in range(B):
        u = K[b] @ wkbq                                   # [1024]
        mb = mask[b, 0].astype(np.float32) * np.float32(-1e9) \
            + np.float32(SM_SCALE) * u
        in_maps.append(
            {
                "qt": pack(Q[b].T),
                "kt": pack(K[b].T),
                "vt": pack(V[b].T),
                "m": pack(m),
                "wvl": pack(wvl),
                "blr2": blr2,
                "mb": np.ascontiguousarray(mb.reshape(NK, P).T),  # [128, 8]
                "ones": np.ones((P, P), dtype=bf),
            }
        )
    return in_maps


def postprocess(results):
    Y = np.stack([np.asarray(results[b]["y"]) for b in range(B)])
    att = np.stack([np.asarray(results[b]["attT"]).T for b in range(B)])
    att_ws = np.broadcast_to(att[:, None], (B, H, SQ, SK))
    return Y, att_ws


def kernel(Q, K, V, mask, Wq, bq, Wk, bk, Wv, bv, Wl, bl):
    nc = get_nc()
    in_maps = prepare_in_maps(Q, K, V, mask, Wq, bq, Wk, bk, Wv, bv, Wl, bl)
    res = run_bass_kernel_spmd(nc, in_maps, list(range(B)))
    return postprocess(res.results)
